# revision 8
# baseline (speedup 1.0000x reference)
"""Embedding lookup (mixed const/trainable tables) on 8 Trainium2 NeuronCores.

Problem (full shapes, fp32):
    X          [524288, 128]   const table (only rows with const_mask==1 are read)
    const_mask [524288]        1 = const row (read from X), 0 = trainable row
    weight     [262144, 128]   trainable table, indexed by rank among mask==0 rows
    index      [262144]        lookup ids into the 524288-row id space
    out        [262144, 128]   out[i] = X[index[i]] if const else weight[var_pos[index[i]]]

Strategy (model parallel, deduplicated, DP-covered, bf16 transport):
    - Host compacts X to its const rows (Xe) so both tables have 262144 rows;
      both are row-sharded over the 8 cores (32768 rows/core/table so local
      row ids fit dma_gather's int16 index format). Tables are staged to the
      device in bf16 (rel err ~4e-3, well inside the 2e-2 gate) which halves
      both HBM read and write traffic.
    - Each lookup routes to the owning (core, table) bucket and is
      DEDUPLICATED (a distinct row is gathered once; duplicates expand in the
      host-side scatter).
    - Per bucket, the sorted distinct rows are covered by variable-size
      descriptors from the tier set TIERS (rows per descriptor; elem_step
      overlap makes a tier-t descriptor read t consecutive table rows as one
      t*256B transfer). Cover is chosen by a DP that trades one descriptor
      (~7ns of Q7 descriptor generation) against junk rows read+written
      across bridged gaps (~1.4ns/covered row of DMA bandwidth at
      ~380GB/s/core), so descriptor-generation time and byte-transfer time
      come out balanced (~65us each per core).
    - Device kernel per core: sync engine loads the merged index streams via
      HWDGE while GPSIMD reloads the Q7 library; then one dma_gather (SWDGE)
      per (bucket, tier) stream HBM->SBUF, each followed by HWDGE writes
      SBUF->HBM, all overlapped. num_idxs registers are compile-time
      immediates (= capacity); the Q7 ucode trims the trailing -1 idx padding
      so each core generates exactly its own descriptor count. Writes cover
      exactly the capacity slots (full columns + partial tail column), not
      the 128-rounded tile, to avoid padding write traffic.
    - Streams are issued big-tier first so the byte-heavy transfers drain
      while small-tier descriptors generate, and the last stream is
      byte-light to shorten the tail.
    - Host scatters the gathered distinct rows back to all lookup positions
      and upcasts to fp32.
"""

import numpy as np
import ml_dtypes

import concourse.bass as bass
import concourse.bacc as bacc
import concourse.mybir as mybir
from concourse.bass_utils import run_bass_kernel_spmd
from concourse.library_config import mlp

NCORES = 8
D = 128             # feature dim -> 256B rows in bf16
SH = 32768          # table rows per core per table (int16 gather index limit)
BF16 = ml_dtypes.bfloat16

# Descriptor tiers: rows covered per descriptor. 64 rows * 256B = 16KB = one
# SDMA packet (bigger would double the per-descriptor M2S packet count).
TIERS = (64, 32, 16, 8, 4, 1)

# Cover DP constants (ns). ALPHA ~ Q7 descriptor-generation cost per
# descriptor; LAM ~ shadow price of one covered row (256B read + 256B
# written), tuned so descriptor-generation time ~= byte time on hardware.
ALPHA = 11.1
LAM = 1.5

_prog_cache = {}


def _stream_list():
    """(name, bucket, tier) per stream, in issue order: big tiers first so
    their transfers drain behind small-tier desc-gen; last stream byte-light."""
    out = []
    for t in TIERS:
        for b in ("X", "W"):
            out.append((f"{b}{t}", b, t))
    return out


def _build_program(caps, regvals):
    """Per-core SPMD bass program: gather streams + exact-capacity writes.

    caps:    per-stream idx capacities (multiples of 16), _stream_list() order
    regvals: per-stream num_idxs register values (immediates). The HW program
             passes caps (ucode trims trailing -1 padding to the per-core
             count); the CoreSim validation program passes exact counts
             (the simulator asserts reg == count).
    """
    nc = bacc.Bacc("TRN2", target_bir_lowering=False)
    streams = _stream_list()
    assert len(caps) == len(streams)

    tabs = {
        "X": nc.dram_tensor("tabX", [SH, D], mybir.dt.bfloat16, kind="ExternalInput"),
        "W": nc.dram_tensor("tabW", [SH, D], mybir.dt.bfloat16, kind="ExternalInput"),
    }
    totc = sum(caps)
    idxall = nc.dram_tensor("idxall", [128, totc // 16], mybir.dt.int16,
                            kind="ExternalInput")
    cnts = nc.dram_tensor("cnts", [128, len(streams)], mybir.dt.int32,
                          kind="ExternalInput")
    outs = {}
    for (nm, b, t), cap in zip(streams, caps):
        outs[nm] = nc.dram_tensor(
            f"out{nm}", [128, -(-cap // 128), t * D], mybir.dt.bfloat16,
            kind="ExternalOutput",
        )

    from contextlib import ExitStack

    with ExitStack() as ctx:
        # write-completion sems already guarantee all DMAs retired; skipping
        # the gpsimd dge_drain removes ~10us from the kernel tail
        block = ctx.enter_context(nc.Block(no_gpsimd_drain=True))
        isb = ctx.enter_context(
            nc.sbuf_tensor("isb", [128, totc // 16], mybir.dt.int16)
        )
        csb = ctx.enter_context(
            nc.sbuf_tensor("csb", [128, len(streams)], mybir.dt.int32)
        )
        tiles, gsem = {}, {}
        for (nm, b, t), cap in zip(streams, caps):
            tiles[nm] = ctx.enter_context(
                nc.sbuf_tensor(f"tile{nm}", [128, -(-cap // 128), t * D],
                               mybir.dt.bfloat16)
            )
            gsem[nm] = ctx.enter_context(nc.semaphore(f"g{nm}"))
        io = ctx.enter_context(nc.semaphore("io"))
        wsem = ctx.enter_context(nc.semaphore("w"))

        n_writes = sum(
            (1 if cap >= 128 else 0) + (1 if cap % 128 else 0) for cap in caps
        )

        @block.gpsimd
        def _(g: bass.BassGpSimd):
            g.load_library(mlp)
            g.wait_ge(io, 32)
            from contextlib import ExitStack as ES
            rctx = ctx.enter_context(ES())
            regs = {
                nm: rctx.enter_context(g.register(f"r{nm}"))
                for nm, *_ in streams
            }
            for i, (nm, *_) in enumerate(streams):
                g.reg_load(regs[nm], csb[0:1, i : i + 1])
            off = 0
            for (nm, b, t), cap, rv in zip(streams, caps, regvals):
                if t > 1:
                    # overlapping view: row stride D, element t*D ->
                    # idx r reads rows r..r+t-1 as one descriptor
                    src = bass.AP(
                        tabs[b], 0, [[D, SH - (t - 1)], [1, t * D]]
                    )
                    step = D
                else:
                    src = tabs[b][:]
                    step = None
                g.dma_gather(
                    tiles[nm][:],
                    src,
                    isb[:, off : off + cap // 16],
                    cap,
                    regs[nm],
                    t * D,
                    elem_step=step,
                    single_packet=False,
                ).then_inc(gsem[nm], 16)
                off += cap // 16

        @block.sync
        def _(s: bass.BassEngine):
            s.dma_start(isb[:], idxall[:]).then_inc(io, 16)
            s.dma_start(csb[:], cnts[:]).then_inc(io, 16)
            for (nm, b, t), cap in zip(streams, caps):
                s.wait_ge(gsem[nm], 16)
                nfull = cap // 128
                rem = cap % 128
                if nfull:
                    s.dma_start(
                        outs[nm][:, :nfull, :], tiles[nm][:, :nfull, :]
                    ).then_inc(wsem, 16)
                if rem:
                    s.dma_start(
                        outs[nm][:rem, nfull : nfull + 1, :],
                        tiles[nm][:rem, nfull : nfull + 1, :],
                    ).then_inc(wsem, 16)
            s.wait_ge(wsem, 16 * n_writes)

    nc.compile()
    return nc


def get_program(caps, regvals):
    key = (tuple(caps), tuple(regvals))
    if key not in _prog_cache:
        _prog_cache[key] = _build_program(*key)
    return _prog_cache[key]


def _wrap_idx(seg, cap):
    """Pack a stream's int16 ids into the [128, cap/16] wrapped+replicated
    layout dma_gather expects (idx j at partition j%16, col j//16, replicated
    for the 8 Q7 cores), -1 padded."""
    pad = np.full(cap, -1, np.int16)
    pad[: seg.size] = seg
    wrapped = pad.reshape(cap // 16, 16).T  # [16, cap/16]
    return np.tile(wrapped, (8, 1))


def _route(cm, idx, n_weight_rows):
    """Deduplicated (bucket, local row) routing.

    Returns (ulocal, counts, inv, const_ids):
      ulocal    local table row per distinct slot, bucket-major, sorted
      counts    [16] distinct rows per bucket (bucket = table*8 + core)
      inv       per-lookup index into the distinct-slot space
      const_ids row ids of X that form the compacted const table
    """
    const_rank = np.cumsum(cm) - 1
    var_pos = np.clip(np.cumsum(1 - cm) - 1, 0, n_weight_rows - 1)
    isc = cm[idx] > 0
    r = np.where(isc, const_rank[idx], var_pos[idx])
    bucket = (~isc).astype(np.int64) * NCORES + (r >> 15)
    key = bucket * SH + (r & (SH - 1))
    uniq, inv = np.unique(key, return_inverse=True)
    counts = np.bincount(uniq // SH, minlength=2 * NCORES)
    ulocal = uniq % SH
    const_ids = np.flatnonzero(cm > 0)
    return ulocal, counts, inv, const_ids


def _cover_dp(u):
    """Cover sorted distinct rows u with TIERS descriptors via a DP that
    minimizes ALPHA*ndesc + LAM*covered_rows.

    Returns (tier_starts, tier_code, slot, off):
      tier_starts  {t: int32 array of descriptor start rows, in emit order}
      tier_code    per element of u: index into TIERS of covering descriptor
      slot         per element: ordinal of the descriptor within its tier
      off          per element: row offset inside the descriptor
    """
    n = u.size
    tiers = TIERS
    nt = len(tiers)
    jt = [np.searchsorted(u, u + t).astype(np.int32) for t in tiers]
    cost = np.empty(n + 1, np.float64)
    cost[n] = 0.0
    choice = np.empty(n, np.int8)
    for i in range(n - 1, -1, -1):
        best = 1e30
        bt = 0
        for ti in range(nt):
            c = ALPHA + LAM * tiers[ti] + cost[jt[ti][i]]
            if c < best:
                best = c
                bt = ti
        cost[i] = best
        choice[i] = bt
    tier_starts = {t: [] for t in tiers}
    tier_code = np.empty(n, np.int8)
    slot = np.empty(n, np.int64)
    off = np.empty(n, np.int64)
    i = 0
    while i < n:
        ti = choice[i]
        t = tiers[ti]
        s = min(int(u[i]), SH - t)
        j = int(jt[ti][i])
        lst = tier_starts[t]
        tier_code[i:j] = ti
        slot[i:j] = len(lst)
        off[i:j] = u[i:j] - s
        lst.append(s)
        i = j
    tier_starts = {t: np.asarray(v, np.int32) for t, v in tier_starts.items()}
    return tier_starts, tier_code, slot, off


def _kernel_numpy(X, cm, weight, idx):
    """Host fallback (used only if structural assumptions break)."""
    var_pos = np.clip(np.cumsum(1 - cm) - 1, 0, weight.shape[0] - 1)
    isc = cm[idx] > 0
    out = np.where(isc[:, None], X[idx], weight[var_pos[idx]])
    return out.astype(np.float32)


def kernel(X, const_mask, weight, index):
    X = np.ascontiguousarray(np.asarray(X), dtype=np.float32)
    weight = np.ascontiguousarray(np.asarray(weight), dtype=np.float32)
    cm = np.asarray(const_mask).astype(np.int64)
    idx = np.asarray(index).astype(np.int64)

    ulocal, counts, inv, const_ids = _route(cm, idx, weight.shape[0])

    structural_ok = (
        X.shape == (524288, 128)
        and weight.shape == (262144, 128)
        and const_ids.size == NCORES * SH
        and weight.shape[0] == NCORES * SH
    )
    if not structural_ok:
        return _kernel_numpy(X, cm, weight, idx)

    starts = np.concatenate([[0], np.cumsum(counts)])
    covers = [_cover_dp(ulocal[starts[b] : starts[b + 1]]) for b in range(16)]

    streams = _stream_list()
    # per-stream ids per core; dummy row-0 descriptor where a core has none
    # (a zero-count gather is undefined), never referenced by reassembly
    ids = {}
    for c in range(NCORES):
        for nm, b, t in streams:
            bkt = (0 if b == "X" else NCORES) + c
            seg = covers[bkt][0][t]
            if seg.size == 0:
                seg = np.zeros(1, np.int32)
            ids[(c, nm)] = seg
    caps = tuple(
        max(-(-max(ids[(c, nm)].size for c in range(NCORES)) // 16) * 16, 16)
        for nm, *_ in streams
    )

    Xe16 = X[const_ids].astype(BF16)   # compacted const table [262144, 128]
    W16 = weight.astype(BF16)

    in_maps = []
    for c in range(NCORES):
        im = {
            "tabX": Xe16[c * SH : (c + 1) * SH],
            "tabW": W16[c * SH : (c + 1) * SH],
        }
        blocks = []
        for i, (nm, b, t) in enumerate(streams):
            blocks.append(_wrap_idx(ids[(c, nm)].astype(np.int16), caps[i]))
        im["idxall"] = np.ascontiguousarray(np.concatenate(blocks, axis=1))
        cvec = np.array([ids[(c, nm)].size for nm, *_ in streams], np.int32)
        im["cnts"] = np.ascontiguousarray(np.tile(cvec, (128, 1)))
        in_maps.append(im)

    nc = get_program(caps, caps)
    res = run_bass_kernel_spmd(nc, in_maps, core_ids=list(range(NCORES)))

    # reassemble: distinct rows bucket-major, then expand duplicates per lookup
    cap_of = {nm: cap for (nm, *_), cap in zip(streams, caps)}
    allrows = np.empty((ulocal.size, D), np.float32)
    for c in range(NCORES):
        for b in ("X", "W"):
            bkt = (0 if b == "X" else NCORES) + c
            tier_starts, tier_code, slot, off = covers[bkt]
            seg = slice(starts[bkt], starts[bkt + 1])
            n = tier_code.size
            arr = np.empty((n, D), np.float32)
            for ti, t in enumerate(TIERS):
                m = tier_code == ti
                if not m.any():
                    continue
                nm = f"{b}{t}"
                cap = cap_of[nm]
                ncols = -(-cap // 128)
                buf = res.results[c][f"out{nm}"].reshape(128, ncols, t, D)
                sl = slot[m]
                arr[m] = buf[sl % 128, sl // 128, off[m], :].astype(np.float32)
            allrows[seg] = arr
    return allrows[inv]


# revision 9
# speedup vs baseline: 1.1936x; 1.1936x over previous
"""Embedding lookup (mixed const/trainable tables) on 8 Trainium2 NeuronCores.

Problem (full shapes, fp32):
    X          [524288, 128]   const table (only rows with const_mask==1 are read)
    const_mask [524288]        1 = const row (read from X), 0 = trainable row
    weight     [262144, 128]   trainable table, indexed by rank among mask==0 rows
    index      [262144]        lookup ids into the 524288-row id space
    out        [262144, 128]   out[i] = X[index[i]] if const else weight[var_pos[index[i]]]

Strategy (model parallel, deduplicated, DP-covered, bf16 transport):
    - Host compacts X to its const rows (Xe) so both tables have 262144 rows;
      both are row-sharded over the 8 cores (32768 rows/core/table so local
      row ids fit dma_gather's int16 index format). Tables are staged to the
      device in bf16 (rel err ~4e-3, well inside the 2e-2 gate) which halves
      both HBM read and write traffic.
    - Each lookup routes to the owning (core, table) bucket and is
      DEDUPLICATED (a distinct row is gathered once; duplicates expand in the
      host-side scatter).
    - Per bucket, the sorted distinct rows are covered by variable-size
      descriptors from the tier set TIERS (rows per descriptor; elem_step
      overlap makes a tier-t descriptor read t consecutive table rows as one
      t*256B transfer). Cover is chosen by a DP that trades one descriptor
      (~7ns of Q7 descriptor generation) against junk rows read+written
      across bridged gaps (~1.4ns/covered row of DMA bandwidth at
      ~380GB/s/core), so descriptor-generation time and byte-transfer time
      come out balanced (~65us each per core).
    - Device kernel per core: sync engine loads the merged index streams via
      HWDGE while GPSIMD reloads the Q7 library; then one dma_gather (SWDGE)
      per (bucket, tier) stream HBM->SBUF, each followed by HWDGE writes
      SBUF->HBM, all overlapped. num_idxs registers are compile-time
      immediates (= capacity); the Q7 ucode trims the trailing -1 idx padding
      so each core generates exactly its own descriptor count. Writes cover
      exactly the capacity slots (full columns + partial tail column), not
      the 128-rounded tile, to avoid padding write traffic.
    - Streams are issued big-tier first so the byte-heavy transfers drain
      while small-tier descriptors generate, and the last stream is
      byte-light to shorten the tail.
    - Host scatters the gathered distinct rows back to all lookup positions
      and upcasts to fp32.
"""

import numpy as np
import ml_dtypes

import concourse.bass as bass
import concourse.bacc as bacc
import concourse.mybir as mybir
from concourse.bass_utils import run_bass_kernel_spmd
from concourse.library_config import mlp

NCORES = 8
D = 128             # feature dim -> 256B rows in bf16
SH = 32768          # table rows per core per table (int16 gather index limit)
BF16 = ml_dtypes.bfloat16

# Descriptor tiers: rows covered per descriptor. 64 rows * 256B = 16KB = one
# SDMA packet (bigger would double the per-descriptor M2S packet count).
TIERS = (64, 16, 8, 4, 1)

# Cover DP constants (ns). ALPHA ~ Q7 descriptor-generation cost per
# descriptor; LAM ~ shadow price of one covered row (256B read + 256B
# written), tuned so descriptor-generation time ~= byte time on hardware.
ALPHA = 6.5
LAM = 2.7

_prog_cache = {}


def _stream_list():
    """(name, bucket, tier) per stream, in issue order: big tiers first so
    their transfers drain behind small-tier desc-gen; last stream byte-light."""
    out = []
    for t in TIERS:
        for b in ("X", "W"):
            out.append((f"{b}{t}", b, t))
    return out


def _build_program(caps, regvals):
    """Per-core SPMD bass program: gather streams + exact-capacity writes.

    caps:    per-stream idx capacities (multiples of 16), _stream_list() order
    regvals: per-stream num_idxs register values (immediates). The HW program
             passes caps (ucode trims trailing -1 padding to the per-core
             count); the CoreSim validation program passes exact counts
             (the simulator asserts reg == count).
    """
    nc = bacc.Bacc("TRN2", target_bir_lowering=False)
    streams = _stream_list()
    assert len(caps) == len(streams)

    tabs = {
        "X": nc.dram_tensor("tabX", [SH, D], mybir.dt.bfloat16, kind="ExternalInput"),
        "W": nc.dram_tensor("tabW", [SH, D], mybir.dt.bfloat16, kind="ExternalInput"),
    }
    totc = sum(caps)
    idxall = nc.dram_tensor("idxall", [128, totc // 16], mybir.dt.int16,
                            kind="ExternalInput")
    cnts = nc.dram_tensor("cnts", [128, len(streams)], mybir.dt.int32,
                          kind="ExternalInput")
    outs = {}
    for (nm, b, t), cap in zip(streams, caps):
        outs[nm] = nc.dram_tensor(
            f"out{nm}", [128, -(-cap // 128), t * D], mybir.dt.bfloat16,
            kind="ExternalOutput",
        )

    from contextlib import ExitStack

    with ExitStack() as ctx:
        # write-completion sems already guarantee all DMAs retired; skipping
        # the gpsimd dge_drain removes ~10us from the kernel tail
        block = ctx.enter_context(nc.Block(no_gpsimd_drain=True))
        isb = ctx.enter_context(
            nc.sbuf_tensor("isb", [128, totc // 16], mybir.dt.int16)
        )
        csb = ctx.enter_context(
            nc.sbuf_tensor("csb", [128, len(streams)], mybir.dt.int32)
        )
        tiles, gsem = {}, {}
        for (nm, b, t), cap in zip(streams, caps):
            tiles[nm] = ctx.enter_context(
                nc.sbuf_tensor(f"tile{nm}", [128, -(-cap // 128), t * D],
                               mybir.dt.bfloat16)
            )
            gsem[nm] = ctx.enter_context(nc.semaphore(f"g{nm}"))
        io = ctx.enter_context(nc.semaphore("io"))
        wsem = ctx.enter_context(nc.semaphore("w"))

        n_writes = sum(
            (1 if cap >= 128 else 0) + (1 if cap % 128 else 0) for cap in caps
        )

        @block.gpsimd
        def _(g: bass.BassGpSimd):
            g.load_library(mlp)
            g.wait_ge(io, 32)
            from contextlib import ExitStack as ES
            rctx = ctx.enter_context(ES())
            regs = {
                nm: rctx.enter_context(g.register(f"r{nm}"))
                for nm, *_ in streams
            }
            for i, (nm, *_) in enumerate(streams):
                g.reg_load(regs[nm], csb[0:1, i : i + 1])
            off = 0
            for (nm, b, t), cap, rv in zip(streams, caps, regvals):
                if t > 1:
                    # overlapping view: row stride D, element t*D ->
                    # idx r reads rows r..r+t-1 as one descriptor
                    src = bass.AP(
                        tabs[b], 0, [[D, SH - (t - 1)], [1, t * D]]
                    )
                    step = D
                else:
                    src = tabs[b][:]
                    step = None
                g.dma_gather(
                    tiles[nm][:],
                    src,
                    isb[:, off : off + cap // 16],
                    cap,
                    regs[nm],
                    t * D,
                    elem_step=step,
                    single_packet=False,
                ).then_inc(gsem[nm], 16)
                off += cap // 16

        @block.sync
        def _(s: bass.BassEngine):
            s.dma_start(isb[:], idxall[:]).then_inc(io, 16)
            s.dma_start(csb[:], cnts[:]).then_inc(io, 16)
            for (nm, b, t), cap in zip(streams, caps):
                s.wait_ge(gsem[nm], 16)
                nfull = cap // 128
                rem = cap % 128
                if nfull:
                    s.dma_start(
                        outs[nm][:, :nfull, :], tiles[nm][:, :nfull, :]
                    ).then_inc(wsem, 16)
                if rem:
                    s.dma_start(
                        outs[nm][:rem, nfull : nfull + 1, :],
                        tiles[nm][:rem, nfull : nfull + 1, :],
                    ).then_inc(wsem, 16)
            s.wait_ge(wsem, 16 * n_writes)

    nc.compile()
    return nc


def get_program(caps, regvals):
    key = (tuple(caps), tuple(regvals))
    if key not in _prog_cache:
        _prog_cache[key] = _build_program(*key)
    return _prog_cache[key]


def _wrap_idx(seg, cap):
    """Pack a stream's int16 ids into the [128, cap/16] wrapped+replicated
    layout dma_gather expects (idx j at partition j%16, col j//16, replicated
    for the 8 Q7 cores), -1 padded."""
    pad = np.full(cap, -1, np.int16)
    pad[: seg.size] = seg
    wrapped = pad.reshape(cap // 16, 16).T  # [16, cap/16]
    return np.tile(wrapped, (8, 1))


def _route(cm, idx, n_weight_rows):
    """Deduplicated (bucket, local row) routing.

    Returns (ulocal, counts, inv, const_ids):
      ulocal    local table row per distinct slot, bucket-major, sorted
      counts    [16] distinct rows per bucket (bucket = table*8 + core)
      inv       per-lookup index into the distinct-slot space
      const_ids row ids of X that form the compacted const table
    """
    const_rank = np.cumsum(cm) - 1
    var_pos = np.clip(np.cumsum(1 - cm) - 1, 0, n_weight_rows - 1)
    isc = cm[idx] > 0
    r = np.where(isc, const_rank[idx], var_pos[idx])
    bucket = (~isc).astype(np.int64) * NCORES + (r >> 15)
    key = bucket * SH + (r & (SH - 1))
    uniq, inv = np.unique(key, return_inverse=True)
    counts = np.bincount(uniq // SH, minlength=2 * NCORES)
    ulocal = uniq % SH
    const_ids = np.flatnonzero(cm > 0)
    return ulocal, counts, inv, const_ids


def _cover_dp(u):
    """Cover sorted distinct rows u with TIERS descriptors via a DP that
    minimizes ALPHA*ndesc + LAM*covered_rows.

    Returns (tier_starts, tier_code, slot, off):
      tier_starts  {t: int32 array of descriptor start rows, in emit order}
      tier_code    per element of u: index into TIERS of covering descriptor
      slot         per element: ordinal of the descriptor within its tier
      off          per element: row offset inside the descriptor
    """
    n = u.size
    tiers = TIERS
    nt = len(tiers)
    jt = [np.searchsorted(u, u + t).astype(np.int32) for t in tiers]
    cost = np.empty(n + 1, np.float64)
    cost[n] = 0.0
    choice = np.empty(n, np.int8)
    for i in range(n - 1, -1, -1):
        best = 1e30
        bt = 0
        for ti in range(nt):
            c = ALPHA + LAM * tiers[ti] + cost[jt[ti][i]]
            if c < best:
                best = c
                bt = ti
        cost[i] = best
        choice[i] = bt
    tier_starts = {t: [] for t in tiers}
    tier_code = np.empty(n, np.int8)
    slot = np.empty(n, np.int64)
    off = np.empty(n, np.int64)
    i = 0
    while i < n:
        ti = choice[i]
        t = tiers[ti]
        s = min(int(u[i]), SH - t)
        j = int(jt[ti][i])
        lst = tier_starts[t]
        tier_code[i:j] = ti
        slot[i:j] = len(lst)
        off[i:j] = u[i:j] - s
        lst.append(s)
        i = j
    tier_starts = {t: np.asarray(v, np.int32) for t, v in tier_starts.items()}
    return tier_starts, tier_code, slot, off


def _kernel_numpy(X, cm, weight, idx):
    """Host fallback (used only if structural assumptions break)."""
    var_pos = np.clip(np.cumsum(1 - cm) - 1, 0, weight.shape[0] - 1)
    isc = cm[idx] > 0
    out = np.where(isc[:, None], X[idx], weight[var_pos[idx]])
    return out.astype(np.float32)


def kernel(X, const_mask, weight, index):
    X = np.ascontiguousarray(np.asarray(X), dtype=np.float32)
    weight = np.ascontiguousarray(np.asarray(weight), dtype=np.float32)
    cm = np.asarray(const_mask).astype(np.int64)
    idx = np.asarray(index).astype(np.int64)

    ulocal, counts, inv, const_ids = _route(cm, idx, weight.shape[0])

    structural_ok = (
        X.shape == (524288, 128)
        and weight.shape == (262144, 128)
        and const_ids.size == NCORES * SH
        and weight.shape[0] == NCORES * SH
    )
    if not structural_ok:
        return _kernel_numpy(X, cm, weight, idx)

    starts = np.concatenate([[0], np.cumsum(counts)])
    covers = [_cover_dp(ulocal[starts[b] : starts[b + 1]]) for b in range(16)]

    streams = _stream_list()
    # per-stream ids per core; dummy row-0 descriptor where a core has none
    # (a zero-count gather is undefined), never referenced by reassembly
    ids = {}
    for c in range(NCORES):
        for nm, b, t in streams:
            bkt = (0 if b == "X" else NCORES) + c
            seg = covers[bkt][0][t]
            if seg.size == 0:
                seg = np.zeros(1, np.int32)
            ids[(c, nm)] = seg
    caps = tuple(
        max(-(-max(ids[(c, nm)].size for c in range(NCORES)) // 16) * 16, 16)
        for nm, *_ in streams
    )

    Xe16 = X[const_ids].astype(BF16)   # compacted const table [262144, 128]
    W16 = weight.astype(BF16)

    in_maps = []
    for c in range(NCORES):
        im = {
            "tabX": Xe16[c * SH : (c + 1) * SH],
            "tabW": W16[c * SH : (c + 1) * SH],
        }
        blocks = []
        for i, (nm, b, t) in enumerate(streams):
            blocks.append(_wrap_idx(ids[(c, nm)].astype(np.int16), caps[i]))
        im["idxall"] = np.ascontiguousarray(np.concatenate(blocks, axis=1))
        cvec = np.array([ids[(c, nm)].size for nm, *_ in streams], np.int32)
        im["cnts"] = np.ascontiguousarray(np.tile(cvec, (128, 1)))
        in_maps.append(im)

    nc = get_program(caps, caps)
    res = run_bass_kernel_spmd(nc, in_maps, core_ids=list(range(NCORES)))

    # reassemble: distinct rows bucket-major, then expand duplicates per lookup
    cap_of = {nm: cap for (nm, *_), cap in zip(streams, caps)}
    allrows = np.empty((ulocal.size, D), np.float32)
    for c in range(NCORES):
        for b in ("X", "W"):
            bkt = (0 if b == "X" else NCORES) + c
            tier_starts, tier_code, slot, off = covers[bkt]
            seg = slice(starts[bkt], starts[bkt + 1])
            n = tier_code.size
            arr = np.empty((n, D), np.float32)
            for ti, t in enumerate(TIERS):
                m = tier_code == ti
                if not m.any():
                    continue
                nm = f"{b}{t}"
                cap = cap_of[nm]
                ncols = -(-cap // 128)
                buf = res.results[c][f"out{nm}"].reshape(128, ncols, t, D)
                sl = slot[m]
                arr[m] = buf[sl % 128, sl // 128, off[m], :].astype(np.float32)
            allrows[seg] = arr
    return allrows[inv]


# revision 10
# speedup vs baseline: 1.2322x; 1.0323x over previous
"""Embedding lookup (mixed const/trainable tables) on 8 Trainium2 NeuronCores.

Problem (full shapes, fp32):
    X          [524288, 128]   const table (only rows with const_mask==1 are read)
    const_mask [524288]        1 = const row (read from X), 0 = trainable row
    weight     [262144, 128]   trainable table, indexed by rank among mask==0 rows
    index      [262144]        lookup ids into the 524288-row id space
    out        [262144, 128]   out[i] = X[index[i]] if const else weight[var_pos[index[i]]]

Strategy (model parallel, deduplicated, DP-covered, bf16 transport):
    - Host compacts X to its const rows (Xe) so both tables have 262144 rows;
      both are row-sharded over the 8 cores (32768 rows/core/table so local
      row ids fit dma_gather's int16 index format). Tables are staged to the
      device in bf16 (rel err ~4e-3, well inside the 2e-2 gate) which halves
      both HBM read and write traffic.
    - Each lookup routes to the owning (core, table) bucket and is
      DEDUPLICATED (a distinct row is gathered once; duplicates expand in the
      host-side scatter).
    - Per bucket, the sorted distinct rows are covered by variable-size
      descriptors from the tier set TIERS (rows per descriptor; elem_step
      overlap makes a tier-t descriptor read t consecutive table rows as one
      t*256B transfer). Cover is chosen by a DP that trades one descriptor
      (~7ns of Q7 descriptor generation) against junk rows read+written
      across bridged gaps (~1.4ns/covered row of DMA bandwidth at
      ~380GB/s/core), so descriptor-generation time and byte-transfer time
      come out balanced (~65us each per core).
    - Device kernel per core: sync engine loads the merged index streams via
      HWDGE while GPSIMD reloads the Q7 library; then one dma_gather (SWDGE)
      per (bucket, tier) stream HBM->SBUF, each followed by HWDGE writes
      SBUF->HBM, all overlapped. num_idxs registers are compile-time
      immediates (= capacity); the Q7 ucode trims the trailing -1 idx padding
      so each core generates exactly its own descriptor count. Writes cover
      exactly the capacity slots (full columns + partial tail column), not
      the 128-rounded tile, to avoid padding write traffic.
    - Streams are issued big-tier first so the byte-heavy transfers drain
      while small-tier descriptors generate, and the last stream is
      byte-light to shorten the tail.
    - Host scatters the gathered distinct rows back to all lookup positions
      and upcasts to fp32.
"""

import numpy as np
import ml_dtypes

import concourse.bass as bass
import concourse.bacc as bacc
import concourse.mybir as mybir
from concourse.bass_utils import run_bass_kernel_spmd
from concourse.library_config import mlp

NCORES = 8
D = 128             # feature dim -> 256B rows in bf16
SH = 32768          # table rows per core per table (int16 gather index limit)
BF16 = ml_dtypes.bfloat16

# Descriptor tiers: rows covered per descriptor. 64 rows * 256B = 16KB = one
# SDMA packet (bigger would double the per-descriptor M2S packet count).
TIERS = (64, 32, 16, 8, 4, 1)

# Cover DP constants (ns). ALPHA ~ Q7 descriptor-generation cost per
# descriptor; LAM ~ shadow price of one covered row (256B read + 256B
# written), tuned so descriptor-generation time ~= byte time on hardware.
ALPHA = 6.5
LAM = 2.9

_prog_cache = {}


def _stream_list():
    """(name, bucket, tier) per stream, in issue order: big tiers first so
    their transfers drain behind small-tier desc-gen; last stream byte-light."""
    out = []
    for t in TIERS:
        for b in ("X", "W"):
            out.append((f"{b}{t}", b, t))
    return out


def _build_program(caps, regvals):
    """Per-core SPMD bass program: gather streams + exact-capacity writes.

    caps:    per-stream idx capacities (multiples of 16), _stream_list() order
    regvals: per-stream num_idxs register values (immediates). The HW program
             passes caps (ucode trims trailing -1 padding to the per-core
             count); the CoreSim validation program passes exact counts
             (the simulator asserts reg == count).
    """
    nc = bacc.Bacc("TRN2", target_bir_lowering=False)
    streams = _stream_list()
    assert len(caps) == len(streams)

    tabs = {
        "X": nc.dram_tensor("tabX", [SH, D], mybir.dt.bfloat16, kind="ExternalInput"),
        "W": nc.dram_tensor("tabW", [SH, D], mybir.dt.bfloat16, kind="ExternalInput"),
    }
    totc = sum(caps)
    idxall = nc.dram_tensor("idxall", [128, totc // 16], mybir.dt.int16,
                            kind="ExternalInput")
    cnts = nc.dram_tensor("cnts", [128, len(streams)], mybir.dt.int32,
                          kind="ExternalInput")
    outs = {}
    for (nm, b, t), cap in zip(streams, caps):
        outs[nm] = nc.dram_tensor(
            f"out{nm}", [128, -(-cap // 128), t * D], mybir.dt.bfloat16,
            kind="ExternalOutput",
        )

    from contextlib import ExitStack

    with ExitStack() as ctx:
        # write-completion sems already guarantee all DMAs retired; skipping
        # the gpsimd dge_drain removes ~10us from the kernel tail
        block = ctx.enter_context(nc.Block(no_gpsimd_drain=True))
        isb = ctx.enter_context(
            nc.sbuf_tensor("isb", [128, totc // 16], mybir.dt.int16)
        )
        csb = ctx.enter_context(
            nc.sbuf_tensor("csb", [128, len(streams)], mybir.dt.int32)
        )
        tiles, gsem = {}, {}
        for (nm, b, t), cap in zip(streams, caps):
            tiles[nm] = ctx.enter_context(
                nc.sbuf_tensor(f"tile{nm}", [128, -(-cap // 128), t * D],
                               mybir.dt.bfloat16)
            )
            gsem[nm] = ctx.enter_context(nc.semaphore(f"g{nm}"))
        io = ctx.enter_context(nc.semaphore("io"))
        wsem = ctx.enter_context(nc.semaphore("w"))

        n_writes = sum(
            (1 if cap >= 128 else 0) + (1 if cap % 128 else 0) for cap in caps
        )

        @block.gpsimd
        def _(g: bass.BassGpSimd):
            g.load_library(mlp)
            g.wait_ge(io, 32)
            from contextlib import ExitStack as ES
            rctx = ctx.enter_context(ES())
            regs = {
                nm: rctx.enter_context(g.register(f"r{nm}"))
                for nm, *_ in streams
            }
            for i, (nm, *_) in enumerate(streams):
                g.reg_load(regs[nm], csb[0:1, i : i + 1])
            off = 0
            for (nm, b, t), cap, rv in zip(streams, caps, regvals):
                if t > 1:
                    # overlapping view: row stride D, element t*D ->
                    # idx r reads rows r..r+t-1 as one descriptor
                    src = bass.AP(
                        tabs[b], 0, [[D, SH - (t - 1)], [1, t * D]]
                    )
                    step = D
                else:
                    src = tabs[b][:]
                    step = None
                g.dma_gather(
                    tiles[nm][:],
                    src,
                    isb[:, off : off + cap // 16],
                    cap,
                    regs[nm],
                    t * D,
                    elem_step=step,
                    single_packet=False,
                ).then_inc(gsem[nm], 16)
                off += cap // 16

        @block.sync
        def _(s: bass.BassEngine):
            s.dma_start(isb[:], idxall[:]).then_inc(io, 16)
            s.dma_start(csb[:], cnts[:]).then_inc(io, 16)
            for (nm, b, t), cap in zip(streams, caps):
                s.wait_ge(gsem[nm], 16)
                nfull = cap // 128
                rem = cap % 128
                if nfull:
                    s.dma_start(
                        outs[nm][:, :nfull, :], tiles[nm][:, :nfull, :]
                    ).then_inc(wsem, 16)
                if rem:
                    s.dma_start(
                        outs[nm][:rem, nfull : nfull + 1, :],
                        tiles[nm][:rem, nfull : nfull + 1, :],
                    ).then_inc(wsem, 16)
            s.wait_ge(wsem, 16 * n_writes)

    nc.compile()
    return nc


def get_program(caps, regvals):
    key = (tuple(caps), tuple(regvals))
    if key not in _prog_cache:
        _prog_cache[key] = _build_program(*key)
    return _prog_cache[key]


def _wrap_idx(seg, cap):
    """Pack a stream's int16 ids into the [128, cap/16] wrapped+replicated
    layout dma_gather expects (idx j at partition j%16, col j//16, replicated
    for the 8 Q7 cores), -1 padded."""
    pad = np.full(cap, -1, np.int16)
    pad[: seg.size] = seg
    wrapped = pad.reshape(cap // 16, 16).T  # [16, cap/16]
    return np.tile(wrapped, (8, 1))


def _route(cm, idx, n_weight_rows):
    """Deduplicated (bucket, local row) routing.

    Returns (ulocal, counts, inv, const_ids):
      ulocal    local table row per distinct slot, bucket-major, sorted
      counts    [16] distinct rows per bucket (bucket = table*8 + core)
      inv       per-lookup index into the distinct-slot space
      const_ids row ids of X that form the compacted const table
    """
    const_rank = np.cumsum(cm) - 1
    var_pos = np.clip(np.cumsum(1 - cm) - 1, 0, n_weight_rows - 1)
    isc = cm[idx] > 0
    r = np.where(isc, const_rank[idx], var_pos[idx])
    bucket = (~isc).astype(np.int64) * NCORES + (r >> 15)
    key = bucket * SH + (r & (SH - 1))
    uniq, inv = np.unique(key, return_inverse=True)
    counts = np.bincount(uniq // SH, minlength=2 * NCORES)
    ulocal = uniq % SH
    const_ids = np.flatnonzero(cm > 0)
    return ulocal, counts, inv, const_ids


def _cover_dp(u):
    """Cover sorted distinct rows u with TIERS descriptors via a DP that
    minimizes ALPHA*ndesc + LAM*covered_rows.

    Returns (tier_starts, tier_code, slot, off):
      tier_starts  {t: int32 array of descriptor start rows, in emit order}
      tier_code    per element of u: index into TIERS of covering descriptor
      slot         per element: ordinal of the descriptor within its tier
      off          per element: row offset inside the descriptor
    """
    n = u.size
    tiers = TIERS
    nt = len(tiers)
    jt = [np.searchsorted(u, u + t).astype(np.int32) for t in tiers]
    cost = np.empty(n + 1, np.float64)
    cost[n] = 0.0
    choice = np.empty(n, np.int8)
    for i in range(n - 1, -1, -1):
        best = 1e30
        bt = 0
        for ti in range(nt):
            c = ALPHA + LAM * tiers[ti] + cost[jt[ti][i]]
            if c < best:
                best = c
                bt = ti
        cost[i] = best
        choice[i] = bt
    tier_starts = {t: [] for t in tiers}
    tier_code = np.empty(n, np.int8)
    slot = np.empty(n, np.int64)
    off = np.empty(n, np.int64)
    i = 0
    while i < n:
        ti = choice[i]
        t = tiers[ti]
        s = min(int(u[i]), SH - t)
        j = int(jt[ti][i])
        lst = tier_starts[t]
        tier_code[i:j] = ti
        slot[i:j] = len(lst)
        off[i:j] = u[i:j] - s
        lst.append(s)
        i = j
    tier_starts = {t: np.asarray(v, np.int32) for t, v in tier_starts.items()}
    return tier_starts, tier_code, slot, off


def _kernel_numpy(X, cm, weight, idx):
    """Host fallback (used only if structural assumptions break)."""
    var_pos = np.clip(np.cumsum(1 - cm) - 1, 0, weight.shape[0] - 1)
    isc = cm[idx] > 0
    out = np.where(isc[:, None], X[idx], weight[var_pos[idx]])
    return out.astype(np.float32)


def kernel(X, const_mask, weight, index):
    X = np.ascontiguousarray(np.asarray(X), dtype=np.float32)
    weight = np.ascontiguousarray(np.asarray(weight), dtype=np.float32)
    cm = np.asarray(const_mask).astype(np.int64)
    idx = np.asarray(index).astype(np.int64)

    ulocal, counts, inv, const_ids = _route(cm, idx, weight.shape[0])

    structural_ok = (
        X.shape == (524288, 128)
        and weight.shape == (262144, 128)
        and const_ids.size == NCORES * SH
        and weight.shape[0] == NCORES * SH
    )
    if not structural_ok:
        return _kernel_numpy(X, cm, weight, idx)

    starts = np.concatenate([[0], np.cumsum(counts)])
    covers = [_cover_dp(ulocal[starts[b] : starts[b + 1]]) for b in range(16)]

    streams = _stream_list()
    # per-stream ids per core; dummy row-0 descriptor where a core has none
    # (a zero-count gather is undefined), never referenced by reassembly
    ids = {}
    for c in range(NCORES):
        for nm, b, t in streams:
            bkt = (0 if b == "X" else NCORES) + c
            seg = covers[bkt][0][t]
            if seg.size == 0:
                seg = np.zeros(1, np.int32)
            ids[(c, nm)] = seg
    caps = tuple(
        max(-(-max(ids[(c, nm)].size for c in range(NCORES)) // 16) * 16, 16)
        for nm, *_ in streams
    )

    Xe16 = X[const_ids].astype(BF16)   # compacted const table [262144, 128]
    W16 = weight.astype(BF16)

    in_maps = []
    for c in range(NCORES):
        im = {
            "tabX": Xe16[c * SH : (c + 1) * SH],
            "tabW": W16[c * SH : (c + 1) * SH],
        }
        blocks = []
        for i, (nm, b, t) in enumerate(streams):
            blocks.append(_wrap_idx(ids[(c, nm)].astype(np.int16), caps[i]))
        im["idxall"] = np.ascontiguousarray(np.concatenate(blocks, axis=1))
        cvec = np.array([ids[(c, nm)].size for nm, *_ in streams], np.int32)
        im["cnts"] = np.ascontiguousarray(np.tile(cvec, (128, 1)))
        in_maps.append(im)

    nc = get_program(caps, caps)
    res = run_bass_kernel_spmd(nc, in_maps, core_ids=list(range(NCORES)))

    # reassemble: distinct rows bucket-major, then expand duplicates per lookup
    cap_of = {nm: cap for (nm, *_), cap in zip(streams, caps)}
    allrows = np.empty((ulocal.size, D), np.float32)
    for c in range(NCORES):
        for b in ("X", "W"):
            bkt = (0 if b == "X" else NCORES) + c
            tier_starts, tier_code, slot, off = covers[bkt]
            seg = slice(starts[bkt], starts[bkt + 1])
            n = tier_code.size
            arr = np.empty((n, D), np.float32)
            for ti, t in enumerate(TIERS):
                m = tier_code == ti
                if not m.any():
                    continue
                nm = f"{b}{t}"
                cap = cap_of[nm]
                ncols = -(-cap // 128)
                buf = res.results[c][f"out{nm}"].reshape(128, ncols, t, D)
                sl = slot[m]
                arr[m] = buf[sl % 128, sl // 128, off[m], :].astype(np.float32)
            allrows[seg] = arr
    return allrows[inv]
